# revision 26
# baseline (speedup 1.0000x reference)
"""DCRNN diffusion-conv GRU cell (single step, zero initial hidden state) on
8 Trainium2 NeuronCores.

Math: with H0 = 0 the reference cell reduces exactly to
    out[b] = sigmoid(-(pre_z)) * tanh(pre_h)
    pre_z  = X Wz00 + Mo Wz01 + Mi Wz11 + bz      (Wg00 = (Wg[0,0]+Wg[1,0])[:128])
    pre_h  = X Wh00 + Mo Wh01 + Mi Wh11 + bh
    Mo = Ao^T X,  Ao[m, n] = sum_{e: src=m, dst=n} coef_o[e]
    Mi = Ai^T X,  Ai[m, n] = sum_{e: dst=m, src=n} coef_i[e]
(R / Wr / br are dead code: H0*R = 0 so Xc2 == Xc.)

Strategy (v3, source-compacted): nodes padded to 5120; core g owns output
nodes [g*640, (g+1)*640) = 5 blocks of 128 for ALL 4 batches.  For each
(matrix d, block lc) "group", only the ~1700 DISTINCT source nodes feeding
that 128-column block matter, so the host compacts them into KMAX=14 chunks
of 128: A_compact[d,lc] is [14*128 src, 128 dst] bf16 and the matching X
rows are host-gathered into Xg[d,lc] = [14*128 src, 4*128 bf] fp8e4m3.
The diffusion product is then a 14-chunk PSUM accumulation per group
(vs 40 chunks block-dense) -- 2.9x fewer PE cycles and 1.2x fewer HBM
bytes than the block-dense v2.  fp8 is used ONLY for the gathered sparse-
path X copies (the mixed bf16 x fp8 matmul keeps A at bf16 precision); the
dense X W00 term runs as a single fp16 matmul per gate.  Overall rel err
~1.4e-2 (< 2e-2 gate; verified bit-exactly in numpy emulation).

Pipeline: groups run lc-major (lc0: Ao,Ai -> lc1: ...); each block's PE
transposes + dense gate matmuls + activations + per-block output DMA
overlap the following block's sparse accumulation, and the last block's
dense epilogue is column-split to shorten the tail.  DMA: the 13.8MB
xg+ac stream rides the single SWDGE queue whole-group in consumption
order (concurrent queues share HBM poorly; SWDGE solo sustains
~350-420 GB/s), with consts + late ac groups on sync and xT + y
writeback on scalar.  ~24 warm-up matmuls during the DMA lead-in open
the HAM clock gate before real work lands.

Measured on 8xTRN2 (axon): ~72-75us vs 188us block-dense baseline.
Tried and rejected (slower on HW): xbar DMA transposes (serialize
against the load stream, +60us), splitting the stream across 2-3
queues (aggregate HBM rate drops), chunk-granular SWDGE pieces (Q7
emission overhead), staging dense X-terms early (DVE adds lengthen
the per-block critical chain).
"""
import math

import numpy as np
import ml_dtypes

import concourse.bacc as bacc
import concourse.tile as tile
from concourse import mybir
from concourse.bass_utils import run_bass_kernel_spmd

P = 128
N_CORES = 8
B = 4
CPC = 5                      # 128-col output blocks per core
KMAX = 14                    # padded source chunks per group
NGRP = 2 * CPC               # groups per core: (lc, d) lc-major
BF16 = ml_dtypes.bfloat16
FP8 = ml_dtypes.float8_e4m3


def _prep(x, edge_index, edge_weight):
    B_, N, F = x.shape
    assert F == P and B_ == B
    npad = math.ceil(N / P / N_CORES) * N_CORES * P      # 5120
    src = edge_index[0].astype(np.int64)
    dst = edge_index[1].astype(np.int64)
    ew = edge_weight.astype(np.float32)

    deg_out = np.bincount(src, weights=ew.astype(np.float64), minlength=N)
    deg_in = np.bincount(dst, weights=ew.astype(np.float64), minlength=N)
    with np.errstate(divide="ignore"):
        dinv_out = np.where(deg_out > 0, 1.0 / deg_out, 0.0).astype(np.float32)
        dinv_in = np.where(deg_in > 0, 1.0 / deg_in, 0.0).astype(np.float32)
    coef = [ew * dinv_out[src], ew * dinv_in[dst]]
    rowcol = [(src, dst), (dst, src)]

    xpad = np.zeros((B, npad, P), np.float32)
    xpad[:, :N] = x
    x8 = xpad.astype(FP8)                                # sparse-path copies
    x16 = xpad.astype(np.float16)                        # dense-path rhs

    per_core = []
    for g in range(N_CORES):
        ac = np.zeros((P, NGRP * KMAX * P), BF16)
        xg = np.zeros((P, NGRP * KMAX * B * P), FP8)
        for lc in range(CPC):
            blk0 = (g * CPC + lc) * P
            for d in range(2):
                G = lc * 2 + d
                rows, cols = rowcol[d]
                sel = (cols >= blk0) & (cols < blk0 + P)
                r, c, w = rows[sel], (cols[sel] - blk0), coef[d][sel]
                uniq, inv = np.unique(r, return_inverse=True)
                K = len(uniq)
                assert K <= KMAX * P, (g, lc, d, K)
                ablk = np.zeros((KMAX * P, P), np.float32)
                np.add.at(ablk, (inv, c), w)
                ac[:, G * KMAX * P:(G + 1) * KMAX * P] = (
                    ablk.reshape(KMAX, P, P).transpose(1, 0, 2)
                    .reshape(P, KMAX * P).astype(BF16))
                upad = np.full(KMAX * P, npad - 1, np.int64)  # zero row
                upad[:K] = uniq
                xr = x8[:, upad, :]                      # [B, KMAX*P, P]
                xg[:, G * KMAX * B * P:(G + 1) * KMAX * B * P] = (
                    xr.transpose(1, 0, 2).reshape(KMAX, P, B, P)
                    .transpose(1, 0, 2, 3).reshape(P, KMAX * B * P))
        # dense-path rhs, lc-major: xT[k, lc*512 + b*128 + j] = x[b, blk0+j, k]
        xs = x16[:, g * CPC * P:(g + 1) * CPC * P, :]    # [B, 640, P]
        xT = np.ascontiguousarray(
            xs.reshape(B, CPC, P, P).transpose(3, 1, 0, 2)
            .reshape(P, CPC * B * P))
        per_core.append({"ac": ac, "xg": xg, "xT": xT})

    meta = dict(B=B, N=N, npad=npad)
    return per_core, meta


def _shared_inputs(Wz, bz, Wh, bh):
    # dense X W00 term runs in fp16 (one matmul); M terms in bf16
    wt16 = np.concatenate([
        (Wz[0, 0][:P] + Wz[1, 0][:P]), (Wh[0, 0][:P] + Wh[1, 0][:P]),
    ], axis=1).astype(np.float16)
    wt = np.concatenate([
        Wz[0, 1][:P].astype(BF16), Wz[1, 1][:P].astype(BF16),
        Wh[0, 1][:P].astype(BF16), Wh[1, 1][:P].astype(BF16),
    ], axis=1)
    bias = np.stack([-bz, bh], axis=1).astype(np.float32)
    ident = np.eye(P, dtype=BF16)
    return wt, wt16, bias, ident


def _build():
    ycols = CPC * B * P                                  # 2560
    bf = mybir.dt.bfloat16
    f8 = mybir.dt.float8e4
    f16 = mybir.dt.float16
    f32 = mybir.dt.float32

    nc = bacc.Bacc("TRN2", target_bir_lowering=False, debug=False,
                   num_devices=N_CORES)
    ac_d = nc.dram_tensor("ac", [P, NGRP * KMAX * P], bf, kind="ExternalInput")
    xg_d = nc.dram_tensor("xg", [P, NGRP * KMAX * B * P], f8,
                          kind="ExternalInput")
    xT_d = nc.dram_tensor("xT", [P, ycols], f16, kind="ExternalInput")
    wt_d = nc.dram_tensor("wt", [P, 4 * P], bf, kind="ExternalInput")
    wt16_d = nc.dram_tensor("wt16", [P, 2 * P], f16, kind="ExternalInput")
    bias_d = nc.dram_tensor("bias", [P, 2], f32, kind="ExternalInput")
    ident_d = nc.dram_tensor("ident", [P, P], bf, kind="ExternalInput")
    yT_d = nc.dram_tensor("yT", [P, ycols], bf, kind="ExternalOutput")

    with tile.TileContext(nc) as tc:
        with (
            tc.tile_pool(name="const", bufs=1) as cpool,
            tc.tile_pool(name="act", bufs=3) as apool,
            tc.tile_pool(name="ps", bufs=3, space="PSUM") as ps_pool,
            tc.tile_pool(name="pt", bufs=3, space="PSUM") as pt_pool,
            tc.tile_pool(name="pd", bufs=2, space="PSUM") as pd_pool,
        ):
            ac_s = cpool.tile([P, NGRP * KMAX * P], bf)
            xg_s = cpool.tile([P, NGRP * KMAX * B * P], f8)
            xT_s = cpool.tile([P, ycols], f16)
            wt_s = cpool.tile([P, 4 * P], bf)
            wt16_s = cpool.tile([P, 2 * P], f16)
            bias_s = cpool.tile([P, 2], f32)
            ident_s = cpool.tile([P, P], bf)
            m_s = [cpool.tile([P, ycols], bf, name=f"m{d}_s") for d in range(2)]
            y_s = cpool.tile([P, ycols], bf)

            # ---- DMA schedule: concurrent queues share HBM poorly, so the
            # whole sparse stream (xg+ac, strictly consumption-ordered) rides
            # the single SWDGE queue at its ~380+ GB/s solo rate.  sync takes
            # the consts + the late ac groups; scalar takes xT (FIRST -- the
            # dense matmuls head-of-line block the PE queue on it) + y out.
            GXB = KMAX * B * P                           # xg cols per group
            GAB = KMAX * P
            nc.sync.dma_start(out=ident_s[:], in_=ident_d[:])
            nc.sync.dma_start(out=bias_s[:], in_=bias_d[:])
            nc.sync.dma_start(out=wt_s[:], in_=wt_d[:])
            nc.sync.dma_start(out=wt16_s[:], in_=wt16_d[:])
            nc.scalar.dma_start(out=xT_s[:], in_=xT_d[:])
            for G in range(NGRP):
                nc.gpsimd.dma_start(out=xg_s[:, G * GXB:(G + 1) * GXB],
                                    in_=xg_d[:, G * GXB:(G + 1) * GXB])
                if G < 6:
                    nc.gpsimd.dma_start(out=ac_s[:, G * GAB:(G + 1) * GAB],
                                        in_=ac_d[:, G * GAB:(G + 1) * GAB])
                else:
                    nc.sync.dma_start(out=ac_s[:, G * GAB:(G + 1) * GAB],
                                      in_=ac_d[:, G * GAB:(G + 1) * GAB])

            # PE warmup/filler: dummy matmuls keep the PE busy (and the HAM
            # clock-gate open) through the front-loaded DMA deficit.  Blocks
            # are interleaved between the early sparse groups so real matmuls
            # run the moment their data lands, at full clock.
            # per gate: (wt16 col, wt col of W01, wt col of W11)
            gate_w = [(0, 0, 1), (1, 2, 3)]              # z, h
            wp = pd_pool.tile([P, P], dtype=f32, name="wp", tag="pd")

            def pe_filler(n):
                for _ in range(n):
                    nc.tensor.matmul(out=wp[:], lhsT=ident_s[:],
                                     rhs=ident_s[:], start=True, stop=True)

            # sized to bridge the full DMA lead-in (~9-20us): first sparse
            # data is never usable before ~20us (per-piece completion floor),
            # and any PE idle >3.4us re-throttles the clock to 1.2GHz.
            pe_filler(120)


            def sparse_group(G):
                pm = ps_pool.tile([P, B * P], dtype=f32, name="pm", tag="ps")
                for c in range(KMAX):
                    nc.tensor.matmul(
                        out=pm[:],
                        lhsT=ac_s[:, (G * KMAX + c) * P:(G * KMAX + c + 1) * P],
                        rhs=xg_s[:, (G * KMAX + c) * B * P:
                                 (G * KMAX + c + 1) * B * P],
                        start=(c == 0),
                        stop=(c == KMAX - 1),
                    )
                return pm

            def sparse_epilogue(lc, pms):
                # psum [n, b*f] -> bf16 -> per-batch PE transpose -> m_s
                for d in range(2):
                    mnm = apool.tile([P, B * P], bf, tag="mnm")
                    nc.vector.tensor_copy(out=mnm[:], in_=pms[d][:])
                    for b in range(B):
                        pt = pt_pool.tile([P, P], dtype=bf, name="pt", tag="pt")
                        nc.tensor.transpose(
                            out=pt[:], in_=mnm[:, b * P:(b + 1) * P],
                            identity=ident_s[:])
                        nc.vector.tensor_copy(
                            out=m_s[d][:, lc * B * P + b * P:
                                       lc * B * P + (b + 1) * P],
                            in_=pt[:])

            def dense_block(lc, nsplit=1):
                W = B * P // nsplit
                for sp in range(nsplit):
                    c0 = lc * B * P + sp * W
                    cs = slice(c0, c0 + W)
                    pz = pd_pool.tile([P, W], dtype=f32, name="pz", tag="pd")
                    ph = pd_pool.tile([P, W], dtype=f32, name="ph", tag="pd")
                    for pt_, (w00, w01, w11) in ((pz, gate_w[0]),
                                                 (ph, gate_w[1])):
                        terms = [(wt16_s, w00, xT_s),
                                 (wt_s, w01, m_s[0]), (wt_s, w11, m_s[1])]
                        for ti, (wtile, wi, rhs_t) in enumerate(terms):
                            nc.tensor.matmul(
                                out=pt_[:],
                                lhsT=wtile[:, wi * P:(wi + 1) * P],
                                rhs=rhs_t[:, cs],
                                start=(ti == 0), stop=(ti == len(terms) - 1))
                    za = apool.tile([P, W], f32, tag="za")
                    nc.scalar.activation(
                        out=za[:], in_=pz[:],
                        func=mybir.ActivationFunctionType.Sigmoid,
                        bias=bias_s[:, 0:1], scale=-1.0)
                    ha = apool.tile([P, W], f32, tag="ha")
                    nc.scalar.activation(
                        out=ha[:], in_=ph[:],
                        func=mybir.ActivationFunctionType.Tanh,
                        bias=bias_s[:, 1:2], scale=1.0)
                    nc.vector.tensor_tensor(
                        out=y_s[:, cs], in0=za[:], in1=ha[:],
                        op=mybir.AluOpType.mult)
                    nc.scalar.dma_start(out=yT_d[:, cs], in_=y_s[:, cs])

            # ---- software pipeline: each block's epilogue follows its two
            # sparse groups immediately; its dense gates follow one group
            # later so the DVE transposed copies land under sparse matmuls.
            for lc in range(CPC):
                pm_o = sparse_group(lc * 2)
                if lc > 0:
                    dense_block(lc - 1)
                pm_i = sparse_group(lc * 2 + 1)
                sparse_epilogue(lc, [pm_o, pm_i])
            dense_block(CPC - 1, nsplit=2)
    nc.compile()
    return nc


def build_all(inputs):
    """Returns (nc, in_maps, meta). Split out so test.py can reuse."""
    x = np.asarray(inputs["x"], np.float32)
    edge_index = np.asarray(inputs["edge_index"])
    edge_weight = np.asarray(inputs["edge_weight"], np.float32)
    Wz = np.asarray(inputs["Wz"], np.float32)
    bz = np.asarray(inputs["bz"], np.float32)
    Wh = np.asarray(inputs["Wh"], np.float32)
    bh = np.asarray(inputs["bh"], np.float32)

    per_core, meta = _prep(x, edge_index, edge_weight)
    wt, wt16, bias, ident = _shared_inputs(Wz, bz, Wh, bh)
    in_maps = []
    for g in range(N_CORES):
        m = dict(per_core[g])
        m["wt"] = wt
        m["wt16"] = wt16
        m["bias"] = bias
        m["ident"] = ident
        in_maps.append(m)
    nc = _build()
    return nc, in_maps, meta


def assemble_output(results, meta):
    B_, N = meta["B"], meta["N"]
    npc = CPC * P
    out = np.empty((B_, N_CORES * npc, P), np.float32)
    for g in range(N_CORES):
        # yT[f, lc*512 + b*128 + j] = out[b, g*640 + lc*128 + j, f]
        blk = (results[g]["yT"].astype(np.float32)
               .reshape(P, CPC, B_, P).transpose(2, 1, 3, 0))
        out[:, g * npc:(g + 1) * npc, :] = blk.reshape(B_, npc, P)
    return np.ascontiguousarray(out[:, :N, :])


def kernel(**inputs) -> np.ndarray:
    nc, in_maps, meta = build_all(inputs)
    res = run_bass_kernel_spmd(nc, in_maps, list(range(N_CORES)))
    return assemble_output(res.results, meta)
